# revision 33
# baseline (speedup 1.0000x reference)
"""Multi-head self-attention (B=8, S=2048, H=256, NH=8, HD=32) on 8 TRN2 cores.

Strategy: data-parallel over batch — each core computes full MHA for one
batch element; no collectives.

Per-core dataflow (all matmuls bf16 in / fp32 PSUM accum):
  - host ships x^T (features on partitions) so no on-device transpose
  - qkT:  q^T,k^T [feat, s] = w_qkv^T @ x — feature-major so each head's
    32 q/k features land on one 32-partition strip
  - scores^T per (head, key-tile): 4 heads computed concurrently via
    4x row-tiled PE (tile_position=(32i,0), K=32)
  - exp on ScalarE straight out of a 4-bank PSUM region ([128,2048] per
    ACTIVATE, scale=1/sqrt(HD) folded in); softmax max-subtraction is
    skipped (scores are O(1), no overflow risk in fp32)
  - ctx^T accumulated over key tiles with 2x column-tiled PE
    (tile_position=(0,0)/(0,64)); stationary v blocks carry a ones
    column so each 64-row tile yields [ctx_h(32) | rowsum(1) | pad]
  - ctx evicted unnormalized to SBUF staging (frees the accumulators for
    the next q-block); the 8 rowsums per q-block are gathered via DRAM,
    one batched VectorE reciprocal, partition-broadcast back via DRAM,
    and multiplied in from staging
  - out = ctxT^T @ w_out_perm + b_out; w_out rows are permuted/zero-padded
    on the host to match the ctxT slot layout
"""
import numpy as np
import ml_dtypes

import bass_rust
import concourse.bass as bass
import concourse.mybir as mybir
import concourse.tile as tile
from concourse.vector_clock import ScopedClock
from concourse.bass_utils import run_bass_kernel_spmd

BF16 = mybir.dt.bfloat16
F32 = mybir.dt.float32
NPBF16 = ml_dtypes.bfloat16

B, S, H = 8, 2048, 256
NH, HD = 8, 32
SCALE = 1.0 / float(np.sqrt(HD))
N_CORES = 8

# Set by a test harness to collect HW timing: {"trace": bool, "trace_cores": [...]}
TRACE_OPTS = {}
LAST_RESULT = None

def _legalize_sync_waits(nc):
    """The walrus build here rejects >1 sync wait per instruction, but Tile
    freely emits 2-3 (and the exit drain up to ~27).  Move excess waits onto
    same-engine NoOp carriers inserted immediately before the offending
    instruction — identical semantics (the engine blocks on each wait in
    program order)."""
    n = 0
    for f in nc.m.functions:
        for bb in f.blocks:
            insts = bb.instructions  # live list
            i = 0
            while i < len(insts):
                inst = insts[i]
                si = inst.sync_info
                if si is not None and len(si.on_wait) > 1:
                    waits = list(si.on_wait)
                    carriers = []
                    for w in waits[:-1]:
                        carriers.append(
                            mybir.InstNoOp(
                                name=f"{inst.name}-w{n}",
                                sync_info=mybir.SyncInfo(on_wait=[w], on_update=[]),
                                bass_nofuse=True,
                                engine=inst.engine,
                            )
                        )
                        n += 1
                    inst.sync_info = bass_rust.SyncInfo(
                        on_wait=waits[-1:], on_update=list(si.on_update)
                    )
                    insts[i:i] = carriers
                    i += len(carriers)
                i += 1
    return n


def _build_nc(legalize=True):
    nc = bass.Bass()
    xt = nc.dram_tensor("xt", [128, 2 * S], BF16, kind="ExternalInput")
    wqk = nc.dram_tensor("wqk", [128, 2 * 512], BF16, kind="ExternalInput")
    bv = nc.dram_tensor("bv", [1, 264], BF16, kind="ExternalInput")
    wv = nc.dram_tensor("wv", [128, 2 * 264], BF16, kind="ExternalInput")
    wo = nc.dram_tensor("wo", [128, 4 * 256], BF16, kind="ExternalInput")
    bqk = nc.dram_tensor("bqk", [1, 512], BF16, kind="ExternalInput")
    ones = nc.dram_tensor("ones", [1, 512], BF16, kind="ExternalInput")
    zrow = nc.dram_tensor("zrow", [2, 2048], BF16, kind="ExternalInput")
    out = nc.dram_tensor("out", [S, H], F32, kind="ExternalOutput")
    # scratch for the partition-broadcast DMA roundtrip (SBUF APs cannot
    # have a zero partition step, DRAM APs can): one row per (qb, pair, side)
    rscr = nc.dram_tensor("rscr", [32, 512], F32)
    rscr2 = nc.dram_tensor("rscr2", [32, 512], F32)

    EXP = mybir.ActivationFunctionType.Exp

    with tile.TileContext(nc) as tc:
        with (
            tc.tile_pool(name="const", bufs=1) as const,
            tc.tile_pool(name="ev", bufs=8) as ev,
            tc.tile_pool(name="etp", bufs=4) as etp,
        ):
            xt_sb = const.tile([128, 2 * S], BF16, tag="xt")
            # split across DMA queues so the load doesn't gate phase 1
            for c in range(4):
                nc.sync.dma_start(
                    out=xt_sb[:, c * S // 2 : (c + 1) * S // 2],
                    in_=xt[:, c * S // 2 : (c + 1) * S // 2],
                )
            wqk_sb = const.tile([128, 2 * 512], BF16, tag="wqk")
            nc.sync.dma_start(out=wqk_sb, in_=wqk[:, :])
            wv_sb = const.tile([128, 2 * 264], BF16, tag="wv")
            nc.sync.dma_start(out=wv_sb, in_=wv[:, :])
            wo_sb = const.tile([128, 4 * 256], BF16, tag="wo")
            nc.sync.dma_start(out=wo_sb, in_=wo[:, :])
            bqk_sb = const.tile([1, 512], BF16, tag="bqk")
            nc.sync.dma_start(out=bqk_sb, in_=bqk[:, :])

            qT_sb = const.tile([128, 2 * S], BF16, tag="qT")
            kT_sb = const.tile([128, 2 * S], BF16, tag="kT")
            v_sb = const.tile([128, 16 * 264], BF16, tag="v")
            ctxT_sb = [
                const.tile([128, S], BF16, tag=f"ctxT{k}", name=f"ctxT{k}")
                for k in range(4)
            ]
            # rows 32:64 / 96:128 of each ctxT tile are never written by the
            # evictions but are contracted by the output matmul (against
            # zeroed w_out rows) — clear them via broadcast DMA so stale NaN
            # patterns can't poison the accumulation
            for k in range(4):
                if k == 0:
                    # row 32 of tile 0 is all-ones: paired with w_out_perm
                    # row 32 = b_out it adds the output bias for free
                    nc.sync.dma_start(out=ctxT_sb[0][32:33, :], in_=zrow[1:2, :])
                    nc.sync.dma_start(
                        out=ctxT_sb[0][33:64, :],
                        in_=zrow[0:1, :].to_broadcast((31, S)),
                    )
                else:
                    nc.sync.dma_start(
                        out=ctxT_sb[k][32:64, :],
                        in_=zrow[0:1, :].to_broadcast((32, S)),
                    )
                nc.sync.dma_start(
                    out=ctxT_sb[k][96:128, :],
                    in_=zrow[0:1, :].to_broadcast((32, S)),
                )

            # ---- phase 0: HAM warmup — ~6µs of dep-free back-to-back
            # matmuls so the PE clock gate opens (1.2 -> 2.4 GHz) before the
            # real work; garbage values land in a scratch PSUM bank that is
            # never read ----
            with tc.tile_pool(name="pp", bufs=4, space="PSUM") as pp:
                warm_sb = const.tile([128, 512], BF16, tag="warm")
                nc.vector.memset(warm_sb, 0.0)
                warm_ps = pp.tile([128, 512], F32, tag="pp")
                for _ in range(12):
                    nc.tensor.matmul(
                        out=warm_ps, lhsT=warm_sb[:, 0:128], rhs=warm_sb[:, :],
                        start=True, stop=True,
                    )

                # ---- phase 1: qT/kT [feature, s] = w_qkv^T @ x; bias folded
                #      into the eviction (per-partition, features-major) ----
                for t in range(4):  # feature tiles: q0,q1,k0,k1
                    for nb in range(4):  # s blocks of 512
                        ps = pp.tile([128, 512], F32, tag="pp")
                        for ks in range(2):
                            nc.tensor.matmul(
                                out=ps,
                                lhsT=wqk_sb[:, ks * 512 + t * 128 : ks * 512 + t * 128 + 128],
                                rhs=xt_sb[:, ks * S + nb * 512 : ks * S + nb * 512 + 512],
                                start=(ks == 0), stop=(ks == 1),
                            )
                        dst = (qT_sb if t < 2 else kT_sb)[
                            :, (t % 2) * S + nb * 512 : (t % 2) * S + nb * 512 + 512
                        ]
                        nc.vector.tensor_scalar_add(
                            out=dst, in0=ps, scalar1=bqkc_sb[:, t : t + 1]
                        )

                # ---- phase 2: v (natural layout, padded 64-wide head slots,
                #      ones column at j=32 for rowsums) ----
                for st in range(16):
                    ps = pp.tile([128, 512], F32, tag="pp")
                    for ks in range(2):
                        nc.tensor.matmul(
                            out=ps,
                            lhsT=xt_sb[:, ks * S + st * 128 : ks * S + st * 128 + 128],
                            rhs=wv_sb[:, ks * 512 : ks * 512 + 512],
                            start=(ks == 0), stop=False,
                        )
                    nc.tensor.matmul(
                        out=ps,
                        lhsT=ones_sb[0:1, 0:128],
                        rhs=bv_sb[0:1, 0:512],
                        start=False, stop=True,
                    )
                    dst = v_sb[:, st * 512 : st * 512 + 512]
                    nc.vector.tensor_copy(out=dst, in_=ps)

            # ---- phase 3: attention, q-blocks of 512 ----
            with (
                tc.tile_pool(name="scp", bufs=2, space="PSUM") as scp,
                tc.tile_pool(name="cxp", bufs=4, space="PSUM") as cxp,
            ):
                for qb in range(4):
                    ctx_tiles = [
                        cxp.tile([128, 512], F32, tag="ctx", name=f"ctx_{qb}_{p}")
                        for p in range(4)
                    ]

                    def emit_ctx(g, kt, eT):
                        # ctx accumulation for (g, kt) — emitted one
                        # iteration late so these PE matmuls fill the window
                        # while ACT runs the *next* exp
                        for pi in range(2):
                            pair = g * 2 + pi
                            cps = ctx_tiles[pair]
                            vc = kt * 264 + pair * 66
                            nc.tensor.matmul(
                                out=cps[0:33, :],
                                lhsT=v_sb[:, vc : vc + 33],
                                rhs=eT[:, (2 * pi) * 512 : (2 * pi) * 512 + 512],
                                start=(kt == 0), stop=(kt == 15),
                                tile_position=(0, 0), skip_group_check=True,
                            )
                            nc.tensor.matmul(
                                out=cps[64:97, :],
                                lhsT=v_sb[:, vc + 33 : vc + 66],
                                rhs=eT[:, (2 * pi + 1) * 512 : (2 * pi + 1) * 512 + 512],
                                start=(kt == 0), stop=(kt == 15),
                                tile_position=(0, 64), skip_group_check=True,
                            )

                    pending = None
                    for kt in range(16):
                        for g in range(2):  # head groups of 4
                            eT = etp.tile([128, 2048], BF16, tag="eT")
                            # two half-groups in separate PSUM tiles: the
                            # half-B exp's WAR doesn't block half-A scores,
                            # so the next scores always overlap the running
                            # exp and ACT never waits on the PE
                            for half in range(2):
                                sc = scp.tile([128, 1024], F32, tag="sc",
                                              name=f"sc_{qb}_{kt}_{g}_{half}")
                                for i in (2 * half, 2 * half + 1):
                                    nc.tensor.matmul(
                                        out=sc[:, (i % 2) * 512 : (i % 2) * 512 + 512],
                                        lhsT=kT_sb[32 * i : 32 * i + 32,
                                                   g * S + kt * 128 : g * S + kt * 128 + 128],
                                        rhs=qT_sb[32 * i : 32 * i + 32,
                                                  g * S + qb * 512 : g * S + qb * 512 + 512],
                                        start=True, stop=True,
                                        tile_position=(32 * i, 0),
                                    )
                                nc.scalar.activation(
                                    out=eT[:, half * 1024 : half * 1024 + 1024],
                                    in_=sc,
                                    func=EXP, scale=SCALE,
                                )
                            if pending is not None:
                                emit_ctx(*pending)
                            pending = (g, kt, eT)
                    emit_ctx(*pending)
                    # Evict unnormalized ctx PSUM -> SBUF staging right away so
                    # the accumulator banks free for the next q-block, then
                    # normalize off the critical path: gather the 8 rowsum
                    # rows via DRAM into one [8,512] tile, one batched
                    # reciprocal (cost ~ free size only), broadcast back
                    # across partitions via DRAM, multiply from staging.
                    stages = []
                    for pair in range(4):
                        stg = ev.tile([128, 512], F32, tag="stg",
                                      name=f"stg_{qb}_{pair}")
                        nc.vector.tensor_copy(
                            out=stg[0:33, :], in_=ctx_tiles[pair][0:33, :]
                        )
                        nc.vector.tensor_copy(
                            out=stg[64:97, :], in_=ctx_tiles[pair][64:97, :]
                        )
                        stages.append(stg)
                        r0 = (qb * 4 + pair) * 2
                        nc.sync.dma_start(out=rscr[r0 : r0 + 1, :], in_=stg[32:33, :])
                        nc.sync.dma_start(out=rscr[r0 + 1 : r0 + 2, :], in_=stg[96:97, :])
                    rsg = ev.tile([8, 512], F32, tag="rsg")
                    nc.sync.dma_start(out=rsg, in_=rscr[qb * 8 : qb * 8 + 8, :])
                    nc.vector.reciprocal(out=rsg, in_=rsg)
                    nc.sync.dma_start(out=rscr2[qb * 8 : qb * 8 + 8, :], in_=rsg)
                    for pair in range(4):
                        stg = stages[pair]
                        rcb = ev.tile([128, 512], F32, tag="rcb",
                                      name=f"rcb_{qb}_{pair}")
                        r0 = (qb * 4 + pair) * 2
                        nc.sync.dma_start(
                            out=rcb[0:32, :],
                            in_=rscr2[r0 : r0 + 1, :].to_broadcast((32, 512)),
                        )
                        nc.sync.dma_start(
                            out=rcb[64:96, :],
                            in_=rscr2[r0 + 1 : r0 + 2, :].to_broadcast((32, 512)),
                        )
                        dst = ctxT_sb[pair]
                        nc.vector.tensor_mul(
                            out=dst[0:32, qb * 512 : qb * 512 + 512],
                            in0=stg[0:32, :], in1=rcb[0:32, :],
                        )
                        nc.vector.tensor_mul(
                            out=dst[64:96, qb * 512 : qb * 512 + 512],
                            in0=stg[64:96, :], in1=rcb[64:96, :],
                        )

            # ---- phase 4: out = ctxT^T @ w_out_perm + b_out ----
            with tc.tile_pool(name="op", bufs=4, space="PSUM") as op:
                for st in range(16):
                    ps = op.tile([128, 256], F32, tag="op")
                    for kk in range(4):
                        nc.tensor.matmul(
                            out=ps,
                            lhsT=ctxT_sb[kk][:, st * 128 : st * 128 + 128],
                            rhs=wo_sb[:, kk * 256 : kk * 256 + 256],
                            start=(kk == 0), stop=(kk == 3),
                        )
                    ot = ev.tile([128, 256], F32, tag="ot")
                    nc.vector.tensor_copy(out=ot, in_=ps)
                    nc.sync.dma_start(
                        out=out[st * 128 : st * 128 + 128, :], in_=ot
                    )
    if legalize:
        _legalize_sync_waits(nc)
    return nc


_NC_CACHE = None


def _get_nc():
    global _NC_CACHE
    if _NC_CACHE is None:
        _NC_CACHE = _build_nc()
    return _NC_CACHE


def _ks_layout(a, nk, cols):
    """[nk*128, cols] -> [128, nk*cols] with [p, k*cols+c] = a[k*128+p, c]."""
    return np.ascontiguousarray(
        a.reshape(nk, 128, cols).transpose(1, 0, 2).reshape(128, nk * cols)
    )


def _prep_in_maps(x, w_qkv, b_qkv, w_out, b_out):
    x = np.asarray(x, dtype=np.float32)
    w_qkv = np.asarray(w_qkv, dtype=np.float32)
    b_qkv = np.asarray(b_qkv, dtype=np.float32)
    w_out = np.asarray(w_out, dtype=np.float32)
    b_out = np.asarray(b_out, dtype=np.float32)

    # shared (per-core identical) weight layouts
    wqk_l = _ks_layout(w_qkv[:, : 2 * H], 2, 512).astype(NPBF16)

    # v weights: 64-wide slot per head: [v_h (32) | ones-col | 31 zero]
    # (the ones column itself is DMA'd on device; v bias is zero per spec)
    wpad = np.zeros((H, 264), np.float32)
    bvr = np.zeros((1, 264), np.float32)
    for h in range(NH):
        c0 = h * 33
        wpad[:, c0 : c0 + 32] = w_qkv[:, 2 * H + h * HD : 2 * H + (h + 1) * HD]
        bvr[0, c0 : c0 + 32] = b_qkv[2 * H + h * HD : 2 * H + (h + 1) * HD]
        bvr[0, c0 + 32] = 1.0  # ones column -> rowsum row
    wv_l = _ks_layout(wpad, 2, 264).astype(NPBF16)


    # w_out rows permuted into the ctxT slot layout (zeros in pad slots)
    wo_perm = np.zeros((512, H), np.float32)
    for pair in range(4):
        for side in range(2):
            h = 2 * pair + side
            r0 = pair * 128 + side * 64
            wo_perm[r0 : r0 + 32, :] = w_out[h * HD : (h + 1) * HD, :]
    wo_perm[32, :] = b_out  # multiplied by the ctxT[0] ones row
    wo_l = _ks_layout(wo_perm, 4, 256).astype(NPBF16)

    shared = {
        "wqk": wqk_l,
        "wv": wv_l,
        "bv": bvr.astype(NPBF16),
        "wo": wo_l,
        "bqk": b_qkv[: 2 * H].astype(NPBF16).reshape(1, 512),
        "ones": np.ones((1, 512), NPBF16),
        "zrow": np.concatenate([np.zeros((1, 2048), NPBF16), np.ones((1, 2048), NPBF16)]),
    }
    in_maps = []
    for b in range(B):
        xt = _ks_layout(np.ascontiguousarray(x[b].T), 2, S).astype(NPBF16)
        in_maps.append({"xt": xt, **shared})
    return in_maps


def kernel(x, w_qkv, b_qkv, w_out, b_out):
    in_maps = _prep_in_maps(x, w_qkv, b_qkv, w_out, b_out)
    nc = _get_nc()
    res = run_bass_kernel_spmd(nc, in_maps, list(range(N_CORES)), **TRACE_OPTS)
    global LAST_RESULT
    LAST_RESULT = res
    return np.stack([res.results[b]["out"] for b in range(B)], axis=0)


# revision 34
# speedup vs baseline: 1.0182x; 1.0182x over previous
"""Multi-head self-attention (B=8, S=2048, H=256, NH=8, HD=32) on 8 TRN2 cores.

Strategy: data-parallel over batch — each core computes full MHA for one
batch element; no collectives.

Per-core dataflow (all matmuls bf16 in / fp32 PSUM accum):
  - host ships x^T (features on partitions) so no on-device transpose
  - qkT:  q^T,k^T [feat, s] = w_qkv^T @ x — feature-major so each head's
    32 q/k features land on one 32-partition strip
  - scores^T per (head, key-tile): 4 heads computed concurrently via
    4x row-tiled PE (tile_position=(32i,0), K=32)
  - exp on ScalarE straight out of a 4-bank PSUM region ([128,2048] per
    ACTIVATE, scale=1/sqrt(HD) folded in); softmax max-subtraction is
    skipped (scores are O(1), no overflow risk in fp32)
  - ctx^T accumulated over key tiles with 2x column-tiled PE
    (tile_position=(0,0)/(0,64)); stationary v blocks carry a ones
    column so each 64-row tile yields [ctx_h(32) | rowsum(1) | pad]
  - ctx evicted unnormalized to SBUF staging (frees the accumulators for
    the next q-block); the 8 rowsums per q-block are gathered via DRAM,
    one batched VectorE reciprocal, partition-broadcast back via DRAM,
    and multiplied in from staging
  - out = ctxT^T @ w_out_perm + b_out; w_out rows are permuted/zero-padded
    on the host to match the ctxT slot layout
"""
import numpy as np
import ml_dtypes

import bass_rust
import concourse.bass as bass
import concourse.mybir as mybir
import concourse.tile as tile
from concourse.vector_clock import ScopedClock
from concourse.bass_utils import run_bass_kernel_spmd

BF16 = mybir.dt.bfloat16
F32 = mybir.dt.float32
NPBF16 = ml_dtypes.bfloat16

B, S, H = 8, 2048, 256
NH, HD = 8, 32
SCALE = 1.0 / float(np.sqrt(HD))
N_CORES = 8

# Set by a test harness to collect HW timing: {"trace": bool, "trace_cores": [...]}
TRACE_OPTS = {}
LAST_RESULT = None

def _legalize_sync_waits(nc):
    """The walrus build here rejects >1 sync wait per instruction, but Tile
    freely emits 2-3 (and the exit drain up to ~27).  Move excess waits onto
    same-engine NoOp carriers inserted immediately before the offending
    instruction — identical semantics (the engine blocks on each wait in
    program order)."""
    n = 0
    for f in nc.m.functions:
        for bb in f.blocks:
            insts = bb.instructions  # live list
            i = 0
            while i < len(insts):
                inst = insts[i]
                si = inst.sync_info
                if si is not None and len(si.on_wait) > 1:
                    waits = list(si.on_wait)
                    carriers = []
                    for w in waits[:-1]:
                        carriers.append(
                            mybir.InstNoOp(
                                name=f"{inst.name}-w{n}",
                                sync_info=mybir.SyncInfo(on_wait=[w], on_update=[]),
                                bass_nofuse=True,
                                engine=inst.engine,
                            )
                        )
                        n += 1
                    inst.sync_info = bass_rust.SyncInfo(
                        on_wait=waits[-1:], on_update=list(si.on_update)
                    )
                    insts[i:i] = carriers
                    i += len(carriers)
                i += 1
    return n


def _build_nc(legalize=True):
    nc = bass.Bass()
    xt = nc.dram_tensor("xt", [128, 2 * S], BF16, kind="ExternalInput")
    wqk = nc.dram_tensor("wqk", [128, 2 * 512], BF16, kind="ExternalInput")
    bv = nc.dram_tensor("bv", [1, 264], BF16, kind="ExternalInput")
    wv = nc.dram_tensor("wv", [128, 2 * 264], BF16, kind="ExternalInput")
    wo = nc.dram_tensor("wo", [128, 4 * 256], BF16, kind="ExternalInput")
    bqk = nc.dram_tensor("bqk", [1, 512], BF16, kind="ExternalInput")
    ones = nc.dram_tensor("ones", [1, 512], BF16, kind="ExternalInput")
    zrow = nc.dram_tensor("zrow", [2, 2048], BF16, kind="ExternalInput")
    out = nc.dram_tensor("out", [S, H], F32, kind="ExternalOutput")
    # scratch for the partition-broadcast DMA roundtrip (SBUF APs cannot
    # have a zero partition step, DRAM APs can): one row per (qb, pair, side)
    rscr = nc.dram_tensor("rscr", [32, 512], F32)
    rscr2 = nc.dram_tensor("rscr2", [32, 512], F32)

    EXP = mybir.ActivationFunctionType.Exp

    with tile.TileContext(nc) as tc:
        with (
            tc.tile_pool(name="const", bufs=1) as const,
            tc.tile_pool(name="ev", bufs=8) as ev,
            tc.tile_pool(name="etp", bufs=4) as etp,
        ):
            xt_sb = const.tile([128, 2 * S], BF16, tag="xt")
            nc.sync.dma_start(out=xt_sb, in_=xt[:, :])
            wqk_sb = const.tile([128, 2 * 512], BF16, tag="wqk")
            nc.sync.dma_start(out=wqk_sb, in_=wqk[:, :])
            wv_sb = const.tile([128, 2 * 264], BF16, tag="wv")
            nc.sync.dma_start(out=wv_sb, in_=wv[:, :])
            wo_sb = const.tile([128, 4 * 256], BF16, tag="wo")
            nc.sync.dma_start(out=wo_sb, in_=wo[:, :])
            bqk_sb = const.tile([1, 512], BF16, tag="bqk")
            nc.sync.dma_start(out=bqk_sb, in_=bqk[:, :])

            qT_sb = const.tile([128, 2 * S], BF16, tag="qT")
            kT_sb = const.tile([128, 2 * S], BF16, tag="kT")
            v_sb = const.tile([128, 16 * 264], BF16, tag="v")
            ctxT_sb = [
                const.tile([128, S], BF16, tag=f"ctxT{k}", name=f"ctxT{k}")
                for k in range(4)
            ]
            # rows 32:64 / 96:128 of each ctxT tile are never written by the
            # evictions but are contracted by the output matmul (against
            # zeroed w_out rows) — clear them via broadcast DMA so stale NaN
            # patterns can't poison the accumulation
            for k in range(4):
                if k == 0:
                    # row 32 of tile 0 is all-ones: paired with w_out_perm
                    # row 32 = b_out it adds the output bias for free
                    nc.sync.dma_start(out=ctxT_sb[0][32:33, :], in_=zrow[1:2, :])
                    nc.sync.dma_start(
                        out=ctxT_sb[0][33:64, :],
                        in_=zrow[0:1, :].to_broadcast((31, S)),
                    )
                else:
                    nc.sync.dma_start(
                        out=ctxT_sb[k][32:64, :],
                        in_=zrow[0:1, :].to_broadcast((32, S)),
                    )
                nc.sync.dma_start(
                    out=ctxT_sb[k][96:128, :],
                    in_=zrow[0:1, :].to_broadcast((32, S)),
                )

            # ---- phase 0: HAM warmup — ~6µs of dep-free back-to-back
            # matmuls so the PE clock gate opens (1.2 -> 2.4 GHz) before the
            # real work; garbage values land in a scratch PSUM bank that is
            # never read ----
            with tc.tile_pool(name="pp", bufs=4, space="PSUM") as pp:
                warm_sb = const.tile([128, 512], BF16, tag="warm")
                nc.vector.memset(warm_sb, 0.0)
                warm_ps = pp.tile([128, 512], F32, tag="pp")
                for _ in range(12):
                    nc.tensor.matmul(
                        out=warm_ps, lhsT=warm_sb[:, 0:128], rhs=warm_sb[:, :],
                        start=True, stop=True,
                    )

                # ---- phase 1: qT/kT [feature, s] = w_qkv^T @ x; bias folded
                #      into the eviction (per-partition, features-major) ----
                for t in range(4):  # feature tiles: q0,q1,k0,k1
                    for nb in range(4):  # s blocks of 512
                        ps = pp.tile([128, 512], F32, tag="pp")
                        for ks in range(2):
                            nc.tensor.matmul(
                                out=ps,
                                lhsT=wqk_sb[:, ks * 512 + t * 128 : ks * 512 + t * 128 + 128],
                                rhs=xt_sb[:, ks * S + nb * 512 : ks * S + nb * 512 + 512],
                                start=(ks == 0), stop=(ks == 1),
                            )
                        dst = (qT_sb if t < 2 else kT_sb)[
                            :, (t % 2) * S + nb * 512 : (t % 2) * S + nb * 512 + 512
                        ]
                        nc.vector.tensor_scalar_add(
                            out=dst, in0=ps, scalar1=bqkc_sb[:, t : t + 1]
                        )

                # ---- phase 2: v (natural layout, padded 64-wide head slots,
                #      ones column at j=32 for rowsums) ----
                for st in range(16):
                    ps = pp.tile([128, 512], F32, tag="pp")
                    for ks in range(2):
                        nc.tensor.matmul(
                            out=ps,
                            lhsT=xt_sb[:, ks * S + st * 128 : ks * S + st * 128 + 128],
                            rhs=wv_sb[:, ks * 512 : ks * 512 + 512],
                            start=(ks == 0), stop=False,
                        )
                    nc.tensor.matmul(
                        out=ps,
                        lhsT=ones_sb[0:1, 0:128],
                        rhs=bv_sb[0:1, 0:512],
                        start=False, stop=True,
                    )
                    dst = v_sb[:, st * 512 : st * 512 + 512]
                    nc.vector.tensor_copy(out=dst, in_=ps)

            # ---- phase 3: attention, q-blocks of 512 ----
            with (
                tc.tile_pool(name="scp", bufs=2, space="PSUM") as scp,
                tc.tile_pool(name="cxp", bufs=4, space="PSUM") as cxp,
            ):
                for qb in range(4):
                    ctx_tiles = [
                        cxp.tile([128, 512], F32, tag="ctx", name=f"ctx_{qb}_{p}")
                        for p in range(4)
                    ]

                    def emit_ctx(g, kt, eT):
                        # ctx accumulation for (g, kt) — emitted one
                        # iteration late so these PE matmuls fill the window
                        # while ACT runs the *next* exp
                        for pi in range(2):
                            pair = g * 2 + pi
                            cps = ctx_tiles[pair]
                            vc = kt * 264 + pair * 66
                            nc.tensor.matmul(
                                out=cps[0:33, :],
                                lhsT=v_sb[:, vc : vc + 33],
                                rhs=eT[:, (2 * pi) * 512 : (2 * pi) * 512 + 512],
                                start=(kt == 0), stop=(kt == 15),
                                tile_position=(0, 0), skip_group_check=True,
                            )
                            nc.tensor.matmul(
                                out=cps[64:97, :],
                                lhsT=v_sb[:, vc + 33 : vc + 66],
                                rhs=eT[:, (2 * pi + 1) * 512 : (2 * pi + 1) * 512 + 512],
                                start=(kt == 0), stop=(kt == 15),
                                tile_position=(0, 64), skip_group_check=True,
                            )

                    pending = None
                    for kt in range(16):
                        for g in range(2):  # head groups of 4
                            eT = etp.tile([128, 2048], BF16, tag="eT")
                            # two half-groups in separate PSUM tiles: the
                            # half-B exp's WAR doesn't block half-A scores,
                            # so the next scores always overlap the running
                            # exp and ACT never waits on the PE
                            for half in range(2):
                                sc = scp.tile([128, 1024], F32, tag="sc",
                                              name=f"sc_{qb}_{kt}_{g}_{half}")
                                for i in (2 * half, 2 * half + 1):
                                    nc.tensor.matmul(
                                        out=sc[:, (i % 2) * 512 : (i % 2) * 512 + 512],
                                        lhsT=kT_sb[32 * i : 32 * i + 32,
                                                   g * S + kt * 128 : g * S + kt * 128 + 128],
                                        rhs=qT_sb[32 * i : 32 * i + 32,
                                                  g * S + qb * 512 : g * S + qb * 512 + 512],
                                        start=True, stop=True,
                                        tile_position=(32 * i, 0),
                                    )
                                nc.scalar.activation(
                                    out=eT[:, half * 1024 : half * 1024 + 1024],
                                    in_=sc,
                                    func=EXP, scale=SCALE,
                                )
                            if pending is not None:
                                emit_ctx(*pending)
                            pending = (g, kt, eT)
                    emit_ctx(*pending)
                    # Evict unnormalized ctx PSUM -> SBUF staging right away so
                    # the accumulator banks free for the next q-block, then
                    # normalize off the critical path: gather the 8 rowsum
                    # rows via DRAM into one [8,512] tile, one batched
                    # reciprocal (cost ~ free size only), broadcast back
                    # across partitions via DRAM, multiply from staging.
                    stages = []
                    for pair in range(4):
                        stg = ev.tile([128, 512], F32, tag="stg",
                                      name=f"stg_{qb}_{pair}")
                        nc.vector.tensor_copy(
                            out=stg[0:33, :], in_=ctx_tiles[pair][0:33, :]
                        )
                        nc.vector.tensor_copy(
                            out=stg[64:97, :], in_=ctx_tiles[pair][64:97, :]
                        )
                        stages.append(stg)
                        r0 = (qb * 4 + pair) * 2
                        nc.sync.dma_start(out=rscr[r0 : r0 + 1, :], in_=stg[32:33, :])
                        nc.sync.dma_start(out=rscr[r0 + 1 : r0 + 2, :], in_=stg[96:97, :])
                    rsg = ev.tile([8, 512], F32, tag="rsg")
                    nc.sync.dma_start(out=rsg, in_=rscr[qb * 8 : qb * 8 + 8, :])
                    nc.vector.reciprocal(out=rsg, in_=rsg)
                    nc.sync.dma_start(out=rscr2[qb * 8 : qb * 8 + 8, :], in_=rsg)
                    for pair in range(4):
                        stg = stages[pair]
                        rcb = ev.tile([128, 512], F32, tag="rcb",
                                      name=f"rcb_{qb}_{pair}")
                        r0 = (qb * 4 + pair) * 2
                        nc.sync.dma_start(
                            out=rcb[0:32, :],
                            in_=rscr2[r0 : r0 + 1, :].to_broadcast((32, 512)),
                        )
                        nc.sync.dma_start(
                            out=rcb[64:96, :],
                            in_=rscr2[r0 + 1 : r0 + 2, :].to_broadcast((32, 512)),
                        )
                        dst = ctxT_sb[pair]
                        nc.vector.tensor_mul(
                            out=dst[0:32, qb * 512 : qb * 512 + 512],
                            in0=stg[0:32, :], in1=rcb[0:32, :],
                        )
                        nc.vector.tensor_mul(
                            out=dst[64:96, qb * 512 : qb * 512 + 512],
                            in0=stg[64:96, :], in1=rcb[64:96, :],
                        )

            # ---- phase 4: out = ctxT^T @ w_out_perm + b_out ----
            with tc.tile_pool(name="op", bufs=4, space="PSUM") as op:
                for st in range(16):
                    ps = op.tile([128, 256], F32, tag="op")
                    for kk in range(4):
                        nc.tensor.matmul(
                            out=ps,
                            lhsT=ctxT_sb[kk][:, st * 128 : st * 128 + 128],
                            rhs=wo_sb[:, kk * 256 : kk * 256 + 256],
                            start=(kk == 0), stop=(kk == 3),
                        )
                    ot = ev.tile([128, 256], F32, tag="ot")
                    nc.vector.tensor_copy(out=ot, in_=ps)
                    nc.sync.dma_start(
                        out=out[st * 128 : st * 128 + 128, :], in_=ot
                    )
    if legalize:
        _legalize_sync_waits(nc)
    return nc


_NC_CACHE = None


def _get_nc():
    global _NC_CACHE
    if _NC_CACHE is None:
        _NC_CACHE = _build_nc()
    return _NC_CACHE


def _ks_layout(a, nk, cols):
    """[nk*128, cols] -> [128, nk*cols] with [p, k*cols+c] = a[k*128+p, c]."""
    return np.ascontiguousarray(
        a.reshape(nk, 128, cols).transpose(1, 0, 2).reshape(128, nk * cols)
    )


def _prep_in_maps(x, w_qkv, b_qkv, w_out, b_out):
    x = np.asarray(x, dtype=np.float32)
    w_qkv = np.asarray(w_qkv, dtype=np.float32)
    b_qkv = np.asarray(b_qkv, dtype=np.float32)
    w_out = np.asarray(w_out, dtype=np.float32)
    b_out = np.asarray(b_out, dtype=np.float32)

    # shared (per-core identical) weight layouts
    wqk_l = _ks_layout(w_qkv[:, : 2 * H], 2, 512).astype(NPBF16)

    # v weights: 64-wide slot per head: [v_h (32) | ones-col | 31 zero]
    # (the ones column itself is DMA'd on device; v bias is zero per spec)
    wpad = np.zeros((H, 264), np.float32)
    bvr = np.zeros((1, 264), np.float32)
    for h in range(NH):
        c0 = h * 33
        wpad[:, c0 : c0 + 32] = w_qkv[:, 2 * H + h * HD : 2 * H + (h + 1) * HD]
        bvr[0, c0 : c0 + 32] = b_qkv[2 * H + h * HD : 2 * H + (h + 1) * HD]
        bvr[0, c0 + 32] = 1.0  # ones column -> rowsum row
    wv_l = _ks_layout(wpad, 2, 264).astype(NPBF16)


    # w_out rows permuted into the ctxT slot layout (zeros in pad slots)
    wo_perm = np.zeros((512, H), np.float32)
    for pair in range(4):
        for side in range(2):
            h = 2 * pair + side
            r0 = pair * 128 + side * 64
            wo_perm[r0 : r0 + 32, :] = w_out[h * HD : (h + 1) * HD, :]
    wo_perm[32, :] = b_out  # multiplied by the ctxT[0] ones row
    wo_l = _ks_layout(wo_perm, 4, 256).astype(NPBF16)

    shared = {
        "wqk": wqk_l,
        "wv": wv_l,
        "bv": bvr.astype(NPBF16),
        "wo": wo_l,
        "bqk": b_qkv[: 2 * H].astype(NPBF16).reshape(1, 512),
        "ones": np.ones((1, 512), NPBF16),
        "zrow": np.concatenate([np.zeros((1, 2048), NPBF16), np.ones((1, 2048), NPBF16)]),
    }
    in_maps = []
    for b in range(B):
        xt = _ks_layout(np.ascontiguousarray(x[b].T), 2, S).astype(NPBF16)
        in_maps.append({"xt": xt, **shared})
    return in_maps


def kernel(x, w_qkv, b_qkv, w_out, b_out):
    in_maps = _prep_in_maps(x, w_qkv, b_qkv, w_out, b_out)
    nc = _get_nc()
    res = run_bass_kernel_spmd(nc, in_maps, list(range(N_CORES)), **TRACE_OPTS)
    global LAST_RESULT
    LAST_RESULT = res
    return np.stack([res.results[b]["out"] for b in range(B)], axis=0)
